# revision 38
# baseline (speedup 1.0000x reference)
"""Trainium2 Bass kernel for nn_COAttention (trilinear co-attention).

Math (per batch, masks are all-ones by problem spec, bias cancels in softmax):
  S    = C@w4C + (Q@w4Q)^T + (C*w4mlu)@Q^T          [Lc, Lq]
  S1   = softmax(S, axis=q) ; S2 = softmax(S, axis=c)
  A    = S1@Q ; Bt = (S1@S2^T)@C = S1@(S2^T@C)      (reassociated)
  out  = concat([C, A, C*A, C*Bt], -1)

Device formulation (single exp pass; exp(sub1) factors cancel in the S2 path):
  E2[c,q] = exp(sub2[c,q] + sub0[c])     (sub0 as per-partition ACT bias)
  w[q]    = exp(sub1[q])                 (host-computed, tiny)
  T'      = (E2^T @ C) / (E2^T @ 1)      == S2^T@C exactly
  F = E2 @ [Q*w | T'*w | w] ; A = F0/r ; Bt = F1/r  (r = F2)

All device inputs are host-packed to DMA-natural [128, ...] layouts so every
load is one full-rate HBM transfer (>=4KB per partition, no xbar, no on-device
input transposes): C^T and Qm^T come straight from the host. E2^T is built by
PE transposes + DVE/Pool PSUM->SBUF copies (which also accumulate the S2
column sums). The two batches per core are software-pipelined: batch b's M3
matmuls interleave with batch b+1's M1/exp stream to keep PE dense.

Host prep (0.05% of FLOPs): sub0=C@w4C, w=exp(Q@w4Q), Qm=Q*w4mlu, Qw=Q*w,
bf16 casts + layout packs. Output: device returns [A|Bt|C*A|C*Bt] bf16 packed
[128, NTC, 512]; host unpacks (dropping the Bt scratch) and prepends exact C.

Sharding: data-parallel over batch, 2 batches per core on 8 cores.
"""

import os
import sys

if "/opt/trn_rl_repo" not in sys.path:
    sys.path.insert(0, "/opt/trn_rl_repo")

import numpy as np
import ml_dtypes

from concourse import bacc, bass, mybir, tile
from concourse.bass_utils import run_bass_kernel_spmd
from concourse.masks import make_identity

F32 = mybir.dt.float32
BF16 = mybir.dt.bfloat16
EXP = mybir.ActivationFunctionType.Exp
COPY = mybir.ActivationFunctionType.Copy
MULT = mybir.AluOpType.mult
ADD = mybir.AluOpType.add
DIV = mybir.AluOpType.divide
AX = mybir.AxisListType.X

B, Lc, Lq, D = 16, 2048, 512, 128
NCORES = 8
BPC = B // NCORES          # batches per core
NTC = Lc // 128            # 16 c-tiles
NTQ = Lq // 128            # 4 q-tiles

_NC_CACHE = {}
LAST_RESULT = None


def _body(tc, nc, INB, INF, OUT):
    with (
        tc.tile_pool(name="const", bufs=1) as constp,
        tc.tile_pool(name="io", bufs=2) as iop,
        tc.tile_pool(name="big", bufs=2) as bigp,
        tc.tile_pool(name="small", bufs=2) as smallp,
        tc.tile_pool(name="ps_s", bufs=2, space="PSUM") as ps_s,
        tc.tile_pool(name="ps_t", bufs=2, space="PSUM") as ps_t,
        tc.tile_pool(name="ps_g", bufs=1, space="PSUM") as ps_g,
        tc.tile_pool(name="ps_f", bufs=3, space="PSUM") as ps_f,
    ):
        ident = constp.tile([128, 128], BF16)
        make_identity(nc, ident[:])

        st = [dict() for _ in range(BPC)]

        def ph_load(b):
            s = st[b]
            # [qmt|ct0:8] first (feeds the first M1 groups), then INF (exp
            # bias), then the rest of ct, then [qwx|cb].
            s["t01"] = iop.tile([128, 20, 128], BF16, tag="t01", name="t01")
            nc.sync.dma_start(s["t01"][:, 0:12, :],
                              INB[b][:, 0:1536].rearrange("p (t d) -> p t d",
                                                          d=128))
            s["sm"] = smallp.tile([128, 20], F32, tag="sm", name="sm")
            nc.sync.dma_start(s["sm"][:], INF[b])
            nc.sync.dma_start(s["t01"][:, 12:20, :],
                              INB[b][:, 1536:2560].rearrange(
                                  "p (t d) -> p t d", d=128))
            s["t23"] = iop.tile([128, 3076], BF16, tag="t23", name="t23")
            nc.sync.dma_start(s["t23"][:], INB[b][:, 2560:5636])
            s["e2n"] = bigp.tile([128, NTC, 512], BF16, tag="e2n", name="e2n")
            s["e2t"] = bigp.tile([128, NTQ, NTC, 128], BF16, tag="e2t",
                                 name="e2t")
            s["spart"] = smallp.tile([128, NTQ, 4], F32, tag="spart",
                                     name="spart")
            s["rr"] = smallp.tile([128, NTC], F32, tag="rr", name="rr")
            s["out_sb"] = bigp.tile([128, NTC, 512], BF16, tag="osb",
                                    name="osb")

        def ph_m1(b, k):
            # S matmuls + exp for c-tiles 4k..4k+3 -> E2 natural [c-part, q]
            s = st[b]
            for m in range(4):
                i = 4 * k + m
                s_ps = ps_s.tile([128, 512], F32, tag="s", name="s")
                nc.tensor.matmul(s_ps[:], lhsT=s["t01"][:, 4 + i, :],
                                 rhs=s["t01"][:, 0:4, :], start=True, stop=True)
                nc.scalar.activation(s["e2n"][:, i, :], s_ps[:], EXP,
                                     bias=s["sm"][:, i : i + 1])

        def ph_tgt(b, k):
            # E2^T via PE transposes + DVE/Pool copies (accumulate col sums),
            # then the Gt accumulation matmuls for the same 4 c-tiles.
            s = st[b]
            if k == 0:
                s["g_ps"] = ps_g.tile([128, 512], F32, tag="g", name="g")
            for j in range(NTQ):
                t_ps = ps_t.tile([128, 4, 128], BF16, tag="t", name="t")
                for m in range(4):
                    i = 4 * k + m
                    nc.tensor.transpose(t_ps[:, m, :],
                                        s["e2n"][:, i, j * 128 : (j + 1) * 128],
                                        ident[:])
                nc.vector.tensor_scalar(
                    out=s["e2t"][:, j, 4 * k : 4 * k + 4, :], in0=t_ps[:],
                    scalar1=1.0, scalar2=None, op0=MULT, op1=ADD,
                    accum_out=s["spart"][:, j, k : k + 1])
            for m in range(4):
                i = 4 * k + m
                cb = s["t23"][:, 1028 + 128 * i : 1028 + 128 * (i + 1)]
                nc.tensor.matmul(s["g_ps"][:], lhsT=cb,
                                 rhs=s["e2n"][:, i, :],
                                 start=(i == 0), stop=(i == NTC - 1))

        def ph_trhs_pe(b):
            # Gt PSUM -> bf16 SBUF (on ACT, so DVE's col-sum/ws chain runs
            # in parallel), then 4 PE transposes -> [q, d]
            s = st[b]
            gt_bf = bigp.tile([128, 512], BF16, tag="gtbf", name="gtbf")
            nc.scalar.activation(gt_bf[:], s["g_ps"][:], COPY)
            s["gt_tp"] = ps_t.tile([128, 4, 128], BF16, tag="t", name="t")
            for j in range(NTQ):
                nc.tensor.transpose(s["gt_tp"][:, j, :],
                                    gt_bf[:, j * 128 : (j + 1) * 128], ident[:])

        def ph_trhs_v(b):
            # col sums -> ws = w/s ; t2 = [T'*w | w]
            s = st[b]
            s_col = smallp.tile([128, NTQ], F32, tag="scol", name="scol")
            nc.vector.reduce_sum(s_col[:], s["spart"][:], axis=AX)
            rs = smallp.tile([128, NTQ], F32, tag="rscol", name="rscol")
            nc.vector.reciprocal(rs[:], s_col[:])
            ws = smallp.tile([128, NTQ], F32, tag="wscol", name="wscol")
            nc.vector.tensor_mul(ws[:], s["sm"][:, 16:20], rs[:])
            for j in range(NTQ):
                # alternate ACT/DVE so T'w slots land in j order ahead of the
                # first M3 group's in-order j consumption
                dst = s["t23"][:, 257 * j + 129 : 257 * (j + 1)]
                if j % 2 == 0:
                    nc.scalar.activation(dst, s["gt_tp"][:, j, :], COPY,
                                         scale=ws[:, j : j + 1])
                else:
                    nc.vector.tensor_scalar_mul(dst, s["gt_tp"][:, j, :],
                                                ws[:, j : j + 1])

        def ph_m3(b, k):
            # F = E2 @ [[w|Qw] | T'w] (contract q) -> f = [r | F0 | F1]; the
            # r column rides the first matmul group so the reciprocal overlaps
            # the second. Posts: A on ACT (scale=rr), fused C*[A|Bt] as one
            # stride-0-broadcast op, alternating DVE/Pool.
            s = st[b]
            for m in range(4):
                i = 4 * k + m
                f_ps = ps_f.tile([128, 257], F32, tag="f", name="f")
                for j in range(NTQ):
                    nc.tensor.matmul(f_ps[:], lhsT=s["e2t"][:, j, i, :],
                                     rhs=s["t23"][:, 257 * j : 257 * (j + 1)],
                                     start=(j == 0), stop=(j == NTQ - 1))
                # DVE recip; [A|Bt] = f[1:257]*rr -> SBUF (ACT/DVE split);
                # [C*A|C*Bt] = [A|Bt] * C (stride-0 in1) on Pool (SBUF-only).
                rr = s["rr"][:, i : i + 1]
                nc.vector.reciprocal(rr, f_ps[:, 0:1])
                if (m % 2 == 0) if b == 0 else (m < 3):
                    nc.scalar.activation(s["out_sb"][:, i, 0:256],
                                         f_ps[:, 1:257], COPY, scale=rr)
                else:
                    nc.vector.tensor_scalar_mul(s["out_sb"][:, i, 0:256],
                                                f_ps[:, 1:257], rr)  # [A|Bt]
                ab = s["out_sb"][:, i, 0:256]
                ab = bass.AP(ab.tensor, ab.offset,
                             [ab.ap[0], [128, 2], [1, 128]])
                cb = s["t23"][:, 1028 + 128 * i : 1028 + 128 * (i + 1)]
                cb2 = bass.AP(cb.tensor, cb.offset,
                              [cb.ap[0], [0, 2], [1, 128]])
                ob = s["out_sb"][:, i, 256:512]
                ob = bass.AP(ob.tensor, ob.offset,
                             [ob.ap[0], [128, 2], [1, 128]])
                eng = nc.gpsimd if m < 3 else nc.vector
                eng.tensor_mul(ob, ab, cb2)                # [C*A|C*Bt]
            if k == 1 or k == 3:
                lo = 0 if k == 1 else 8
                nc.sync.dma_start(OUT[b][:, lo : lo + 8, :],
                                  s["out_sb"][:, lo : lo + 8, :])

        # software pipeline: b0 M1/exp stream first (M1 runs a group ahead so
        # PE queues work instead of stalling on exp); b1's M1/tgt groups then
        # interleave with b0's M3 groups on PE; b1 M3 is the epilogue.
        ph_load(0)
        ph_load(1)
        ph_m1(0, 0)
        ph_m1(0, 1)
        for k in range(4):
            ph_tgt(0, k)
            if k + 2 < 4:
                ph_m1(0, k + 2)
        ph_trhs_pe(0)
        for k in range(4):
            ph_m1(1, k)
            if k == 0:
                ph_trhs_v(0)
            ph_tgt(1, k)
            if k == 3:
                ph_trhs_pe(1)
                ph_trhs_v(1)
            ph_m3(0, k)
        for k in range(4):
            ph_m3(1, k)


def _build_nc(n_iters=1):
    nc = bacc.Bacc("TRN2", target_bir_lowering=False, debug=False)
    INB = nc.declare_dram_parameter("INB_bf", [BPC, 128, 5636], BF16,
                                    isOutput=False)
    INF = nc.declare_dram_parameter("INF_f", [BPC, 128, 20], F32,
                                    isOutput=False)
    OUT = nc.declare_dram_parameter("OUT", [BPC, 128, NTC, 4 * D], BF16,
                                    isOutput=True)
    with tile.TileContext(nc) as tc:
        if n_iters == 1:
            _body(tc, nc, INB, INF, OUT)
        else:
            hints = (mybir.EngineType.PE, mybir.EngineType.DVE,
                     mybir.EngineType.Activation, mybir.EngineType.Pool,
                     mybir.EngineType.SP)
            with tc.For_i(0, n_iters, 1, hint_engines=hints):
                _body(tc, nc, INB, INF, OUT)
    nc.compile()
    return nc


def get_nc():
    if "nc" not in _NC_CACHE:
        _NC_CACHE["nc"] = _build_nc()
    return _NC_CACHE["nc"]


def prep_in_maps(C, Q, w4C, w4Q, w4mlu):
    """Host prep: rank-1 bias terms, input scalings, DMA-natural packing."""
    bf = ml_dtypes.bfloat16
    C = np.asarray(C, dtype=np.float32)
    Q = np.asarray(Q, dtype=np.float32)
    w4C = np.asarray(w4C, dtype=np.float32).reshape(D)
    w4Q = np.asarray(w4Q, dtype=np.float32).reshape(D)
    w4mlu = np.asarray(w4mlu, dtype=np.float32).reshape(D)

    sub0 = C @ w4C                                   # [B, Lc]
    w = np.exp(Q @ w4Q)                              # [B, Lq]
    Qm = Q * w4mlu                                   # [B, Lq, D]
    Qw = Q * w[:, :, None]                           # [B, Lq, D]

    qmt = Qm.transpose(0, 2, 1)                                  # [B,128,512]
    ct = C.transpose(0, 2, 1)                                    # [B,128,2048]
    qwx = np.concatenate(
        [w.reshape(B, NTQ, 128).transpose(0, 2, 1)[:, :, :, None],
         Qw.reshape(B, NTQ, 128, D).transpose(0, 2, 1, 3),
         np.zeros((B, 128, NTQ, D), np.float32)],
        axis=3).reshape(B, 128, NTQ * 257)      # [w|Qw|T'w-slot] per q-tile
    cb = C.reshape(B, NTC, 128, D).transpose(0, 2, 1, 3).reshape(B, 128, 2048)
    INB = np.concatenate([qmt, ct, qwx, cb], axis=2).astype(bf)  # [B,128,5124]
    INF = np.concatenate([sub0.reshape(B, NTC, 128).transpose(0, 2, 1),
                          w.reshape(B, NTQ, 128).transpose(0, 2, 1)],
                         axis=2).astype(np.float32)              # [B,128,20]
    INB = np.ascontiguousarray(INB)
    INF = np.ascontiguousarray(INF)

    in_maps = []
    for k in range(NCORES):
        sl = slice(k * BPC, (k + 1) * BPC)
        in_maps.append({
            "INB_bf": np.ascontiguousarray(INB[sl]),
            "INF_f": np.ascontiguousarray(INF[sl]),
        })
    return in_maps


def kernel(C, Q, Cmask=None, Qmask=None, w4C=None, w4Q=None, w4mlu=None,
           bias=None, **_unused):
    """Full inputs in, full output out. Masks are all-ones (problem spec);
    bias is a scalar added to S pre-softmax, which cancels in both softmaxes."""
    global LAST_RESULT
    C = np.asarray(C, dtype=np.float32)
    in_maps = prep_in_maps(C, Q, w4C, w4Q, w4mlu)

    nc = get_nc()
    trace = bool(int(os.environ.get("BASS_KERNEL_TRACE", "0")))
    res = run_bass_kernel_spmd(nc, in_maps, list(range(NCORES)), trace=trace)
    LAST_RESULT = res

    # device OUT is [BPC, 128, NTC, 512] = [A|Bt|C*A|C*Bt]; row c = i*128 + p
    # lives at [p, i, :]; Bt is scratch (dropped here).
    acb = np.concatenate(
        [np.asarray(res.results[k]["OUT"]) for k in range(NCORES)],
        axis=0).astype(np.float32)                       # [B, 128, NTC, 512]
    acb = acb.transpose(0, 2, 1, 3).reshape(B, Lc, 4 * D)
    out = np.empty((B, Lc, 4 * D), dtype=np.float32)
    out[..., 0:D] = C
    out[..., D : 2 * D] = acb[..., 0:D]
    out[..., 2 * D :] = acb[..., 2 * D :]
    return out


# revision 39
# speedup vs baseline: 1.0566x; 1.0566x over previous
"""Trainium2 Bass kernel for nn_COAttention (trilinear co-attention).

Math (per batch, masks are all-ones by problem spec, bias cancels in softmax):
  S    = C@w4C + (Q@w4Q)^T + (C*w4mlu)@Q^T          [Lc, Lq]
  S1   = softmax(S, axis=q) ; S2 = softmax(S, axis=c)
  A    = S1@Q ; Bt = (S1@S2^T)@C = S1@(S2^T@C)      (reassociated)
  out  = concat([C, A, C*A, C*Bt], -1)

Device formulation (single exp pass; exp(sub1) factors cancel in the S2 path):
  E2[c,q] = exp(sub2[c,q] + sub0[c])     (sub0 as per-partition ACT bias)
  w[q]    = exp(sub1[q])                 (host-computed, tiny)
  T'      = (E2^T @ C) / (E2^T @ 1)      == S2^T@C exactly
  F = E2 @ [Q*w | T'*w | w] ; A = F0/r ; Bt = F1/r  (r = F2)

All device inputs are host-packed to DMA-natural [128, ...] layouts so every
load is one full-rate HBM transfer (>=4KB per partition, no xbar, no on-device
input transposes): C^T and Qm^T come straight from the host. E2^T is built by
PE transposes + DVE/Pool PSUM->SBUF copies (which also accumulate the S2
column sums). The two batches per core are software-pipelined: batch b's M3
matmuls interleave with batch b+1's M1/exp stream to keep PE dense.

Host prep (0.05% of FLOPs): sub0=C@w4C, w=exp(Q@w4Q), Qm=Q*w4mlu, Qw=Q*w,
bf16 casts + layout packs. Output: device returns [A|Bt|C*A|C*Bt] bf16 packed
[128, NTC, 512]; host unpacks (dropping the Bt scratch) and prepends exact C.

Sharding: data-parallel over batch, 2 batches per core on 8 cores.
"""

import os
import sys

if "/opt/trn_rl_repo" not in sys.path:
    sys.path.insert(0, "/opt/trn_rl_repo")

import numpy as np
import ml_dtypes

from concourse import bacc, bass, mybir, tile
from concourse.bass_utils import run_bass_kernel_spmd
from concourse.masks import make_identity

F32 = mybir.dt.float32
BF16 = mybir.dt.bfloat16
EXP = mybir.ActivationFunctionType.Exp
COPY = mybir.ActivationFunctionType.Copy
MULT = mybir.AluOpType.mult
ADD = mybir.AluOpType.add
DIV = mybir.AluOpType.divide
AX = mybir.AxisListType.X

B, Lc, Lq, D = 16, 2048, 512, 128
NCORES = 8
BPC = B // NCORES          # batches per core
NTC = Lc // 128            # 16 c-tiles
NTQ = Lq // 128            # 4 q-tiles

_NC_CACHE = {}
LAST_RESULT = None


def _body(tc, nc, INB, INF, OUT):
    with (
        tc.tile_pool(name="const", bufs=1) as constp,
        tc.tile_pool(name="io", bufs=2) as iop,
        tc.tile_pool(name="big", bufs=2) as bigp,
        tc.tile_pool(name="small", bufs=2) as smallp,
        tc.tile_pool(name="ps_s", bufs=2, space="PSUM") as ps_s,
        tc.tile_pool(name="ps_t", bufs=2, space="PSUM") as ps_t,
        tc.tile_pool(name="ps_g", bufs=1, space="PSUM") as ps_g,
        tc.tile_pool(name="ps_f", bufs=3, space="PSUM") as ps_f,
    ):
        ident = constp.tile([128, 128], BF16)
        make_identity(nc, ident[:])

        st = [dict() for _ in range(BPC)]

        def ph_load(b):
            s = st[b]
            # [qmt|ct0:8] first (feeds the first M1 groups), then INF (exp
            # bias), then the rest of ct, then [qwx|cb].
            s["t01"] = iop.tile([128, 20, 128], BF16, tag="t01", name="t01")
            nc.sync.dma_start(s["t01"][:, 0:12, :],
                              INB[b][:, 0:1536].rearrange("p (t d) -> p t d",
                                                          d=128))
            s["sm"] = smallp.tile([128, 20], F32, tag="sm", name="sm")
            nc.sync.dma_start(s["sm"][:], INF[b])
            nc.sync.dma_start(s["t01"][:, 12:20, :],
                              INB[b][:, 1536:2560].rearrange(
                                  "p (t d) -> p t d", d=128))
            s["t23"] = iop.tile([128, 3076], BF16, tag="t23", name="t23")
            nc.sync.dma_start(s["t23"][:], INB[b][:, 2560:5636])
            s["e2n"] = bigp.tile([128, NTC, 512], BF16, tag="e2n", name="e2n")
            s["e2t"] = bigp.tile([128, NTQ, NTC, 128], BF16, tag="e2t",
                                 name="e2t")
            s["spart"] = smallp.tile([128, NTQ, 4], F32, tag="spart",
                                     name="spart")
            s["rr"] = smallp.tile([128, NTC], F32, tag="rr", name="rr")
            s["out_sb"] = bigp.tile([128, NTC, 512], BF16, tag="osb",
                                    name="osb")

        def ph_m1(b, k):
            # S matmuls + exp for c-tiles 4k..4k+3 -> E2 natural [c-part, q]
            s = st[b]
            for m in range(4):
                i = 4 * k + m
                s_ps = ps_s.tile([128, 512], F32, tag="s", name="s")
                nc.tensor.matmul(s_ps[:], lhsT=s["t01"][:, 4 + i, :],
                                 rhs=s["t01"][:, 0:4, :], start=True, stop=True)
                nc.scalar.activation(s["e2n"][:, i, :], s_ps[:], EXP,
                                     bias=s["sm"][:, i : i + 1])

        def ph_tgt(b, k):
            # E2^T via PE transposes + DVE/Pool copies (accumulate col sums),
            # then the Gt accumulation matmuls for the same 4 c-tiles.
            s = st[b]
            if k == 0:
                s["g_ps"] = ps_g.tile([128, 512], F32, tag="g", name="g")
            for j in range(NTQ):
                t_ps = ps_t.tile([128, 4, 128], BF16, tag="t", name="t")
                for m in range(4):
                    i = 4 * k + m
                    nc.tensor.transpose(t_ps[:, m, :],
                                        s["e2n"][:, i, j * 128 : (j + 1) * 128],
                                        ident[:])
                nc.vector.tensor_scalar(
                    out=s["e2t"][:, j, 4 * k : 4 * k + 4, :], in0=t_ps[:],
                    scalar1=1.0, scalar2=None, op0=MULT, op1=ADD,
                    accum_out=s["spart"][:, j, k : k + 1])
            for m in range(4):
                i = 4 * k + m
                cb = s["t23"][:, 1028 + 128 * i : 1028 + 128 * (i + 1)]
                nc.tensor.matmul(s["g_ps"][:], lhsT=cb,
                                 rhs=s["e2n"][:, i, :],
                                 start=(i == 0), stop=(i == NTC - 1))

        def ph_trhs_pe(b):
            # Gt PSUM -> bf16 SBUF (on ACT, so DVE's col-sum/ws chain runs
            # in parallel), then 4 PE transposes -> [q, d]
            s = st[b]
            gt_bf = bigp.tile([128, 512], BF16, tag="gtbf", name="gtbf")
            nc.scalar.activation(gt_bf[:], s["g_ps"][:], COPY)
            s["gt_tp"] = ps_t.tile([128, 4, 128], BF16, tag="t", name="t")
            for j in range(NTQ):
                nc.tensor.transpose(s["gt_tp"][:, j, :],
                                    gt_bf[:, j * 128 : (j + 1) * 128], ident[:])

        def ph_trhs_v(b):
            # col sums -> ws = w/s ; t2 = [T'*w | w]
            s = st[b]
            s_col = smallp.tile([128, NTQ], F32, tag="scol", name="scol")
            nc.vector.reduce_sum(s_col[:], s["spart"][:], axis=AX)
            rs = smallp.tile([128, NTQ], F32, tag="rscol", name="rscol")
            nc.vector.reciprocal(rs[:], s_col[:])
            ws = smallp.tile([128, NTQ], F32, tag="wscol", name="wscol")
            nc.vector.tensor_mul(ws[:], s["sm"][:, 16:20], rs[:])
            for j in range(NTQ):
                # alternate ACT/DVE so T'w slots land in j order ahead of the
                # first M3 group's in-order j consumption
                dst = s["t23"][:, 257 * j + 129 : 257 * (j + 1)]
                if j % 2 == 0:
                    nc.scalar.activation(dst, s["gt_tp"][:, j, :], COPY,
                                         scale=ws[:, j : j + 1])
                else:
                    nc.vector.tensor_scalar_mul(dst, s["gt_tp"][:, j, :],
                                                ws[:, j : j + 1])

        def ph_m3(b, k):
            # F = E2 @ [[w|Qw] | T'w] (contract q) -> f = [r | F0 | F1]; the
            # r column rides the first matmul group so the reciprocal overlaps
            # the second. Posts: A on ACT (scale=rr), fused C*[A|Bt] as one
            # stride-0-broadcast op, alternating DVE/Pool.
            s = st[b]
            for m in range(4):
                i = 4 * k + m
                f_ps = ps_f.tile([128, 257], F32, tag="f", name="f")
                for j in range(NTQ):
                    nc.tensor.matmul(f_ps[:], lhsT=s["e2t"][:, j, i, :],
                                     rhs=s["t23"][:, 257 * j : 257 * (j + 1)],
                                     start=(j == 0), stop=(j == NTQ - 1))
                # DVE recip; [A|Bt] = f[1:257]*rr -> SBUF (ACT/DVE split);
                # [C*A|C*Bt] = [A|Bt] * C (stride-0 in1) on Pool (SBUF-only).
                rr = s["rr"][:, i : i + 1]
                nc.vector.reciprocal(rr, f_ps[:, 0:1])
                if (m % 2 == 0) if b == 0 else (m < 3):
                    nc.scalar.activation(s["out_sb"][:, i, 0:256],
                                         f_ps[:, 1:257], COPY, scale=rr)
                else:
                    nc.vector.tensor_scalar_mul(s["out_sb"][:, i, 0:256],
                                                f_ps[:, 1:257], rr)  # [A|Bt]
                ab = s["out_sb"][:, i, 0:256]
                ab = bass.AP(ab.tensor, ab.offset,
                             [ab.ap[0], [128, 2], [1, 128]])
                cb = s["t23"][:, 1028 + 128 * i : 1028 + 128 * (i + 1)]
                cb2 = bass.AP(cb.tensor, cb.offset,
                              [cb.ap[0], [0, 2], [1, 128]])
                ob = s["out_sb"][:, i, 256:512]
                ob = bass.AP(ob.tensor, ob.offset,
                             [ob.ap[0], [128, 2], [1, 128]])
                eng = nc.gpsimd if m < 3 else nc.vector
                eng.tensor_mul(ob, ab, cb2)                # [C*A|C*Bt]
            nc.sync.dma_start(OUT[b][:, 4 * k : 4 * k + 4, :],
                              s["out_sb"][:, 4 * k : 4 * k + 4, :])

        # software pipeline: b0 M1/exp stream first (M1 runs a group ahead so
        # PE queues work instead of stalling on exp); b1's M1/tgt groups then
        # interleave with b0's M3 groups on PE; b1 M3 is the epilogue.
        ph_load(0)
        ph_load(1)
        ph_m1(0, 0)
        ph_m1(0, 1)
        for k in range(4):
            ph_tgt(0, k)
            if k + 2 < 4:
                ph_m1(0, k + 2)
        ph_trhs_pe(0)
        for k in range(4):
            ph_m1(1, k)
            if k == 0:
                ph_trhs_v(0)
            ph_tgt(1, k)
            if k == 3:
                ph_trhs_pe(1)
                ph_trhs_v(1)
            ph_m3(0, k)
        for k in range(4):
            ph_m3(1, k)


def _build_nc(n_iters=1):
    nc = bacc.Bacc("TRN2", target_bir_lowering=False, debug=False)
    INB = nc.declare_dram_parameter("INB_bf", [BPC, 128, 5636], BF16,
                                    isOutput=False)
    INF = nc.declare_dram_parameter("INF_f", [BPC, 128, 20], F32,
                                    isOutput=False)
    OUT = nc.declare_dram_parameter("OUT", [BPC, 128, NTC, 4 * D], BF16,
                                    isOutput=True)
    with tile.TileContext(nc) as tc:
        if n_iters == 1:
            _body(tc, nc, INB, INF, OUT)
        else:
            hints = (mybir.EngineType.PE, mybir.EngineType.DVE,
                     mybir.EngineType.Activation, mybir.EngineType.Pool,
                     mybir.EngineType.SP)
            with tc.For_i(0, n_iters, 1, hint_engines=hints):
                _body(tc, nc, INB, INF, OUT)
    nc.compile()
    return nc


def get_nc():
    if "nc" not in _NC_CACHE:
        _NC_CACHE["nc"] = _build_nc()
    return _NC_CACHE["nc"]


def prep_in_maps(C, Q, w4C, w4Q, w4mlu):
    """Host prep: rank-1 bias terms, input scalings, DMA-natural packing."""
    bf = ml_dtypes.bfloat16
    C = np.asarray(C, dtype=np.float32)
    Q = np.asarray(Q, dtype=np.float32)
    w4C = np.asarray(w4C, dtype=np.float32).reshape(D)
    w4Q = np.asarray(w4Q, dtype=np.float32).reshape(D)
    w4mlu = np.asarray(w4mlu, dtype=np.float32).reshape(D)

    sub0 = C @ w4C                                   # [B, Lc]
    w = np.exp(Q @ w4Q)                              # [B, Lq]
    Qm = Q * w4mlu                                   # [B, Lq, D]
    Qw = Q * w[:, :, None]                           # [B, Lq, D]

    qmt = Qm.transpose(0, 2, 1)                                  # [B,128,512]
    ct = C.transpose(0, 2, 1)                                    # [B,128,2048]
    qwx = np.concatenate(
        [w.reshape(B, NTQ, 128).transpose(0, 2, 1)[:, :, :, None],
         Qw.reshape(B, NTQ, 128, D).transpose(0, 2, 1, 3),
         np.zeros((B, 128, NTQ, D), np.float32)],
        axis=3).reshape(B, 128, NTQ * 257)      # [w|Qw|T'w-slot] per q-tile
    cb = C.reshape(B, NTC, 128, D).transpose(0, 2, 1, 3).reshape(B, 128, 2048)
    INB = np.concatenate([qmt, ct, qwx, cb], axis=2).astype(bf)  # [B,128,5124]
    INF = np.concatenate([sub0.reshape(B, NTC, 128).transpose(0, 2, 1),
                          w.reshape(B, NTQ, 128).transpose(0, 2, 1)],
                         axis=2).astype(np.float32)              # [B,128,20]
    INB = np.ascontiguousarray(INB)
    INF = np.ascontiguousarray(INF)

    in_maps = []
    for k in range(NCORES):
        sl = slice(k * BPC, (k + 1) * BPC)
        in_maps.append({
            "INB_bf": np.ascontiguousarray(INB[sl]),
            "INF_f": np.ascontiguousarray(INF[sl]),
        })
    return in_maps


def kernel(C, Q, Cmask=None, Qmask=None, w4C=None, w4Q=None, w4mlu=None,
           bias=None, **_unused):
    """Full inputs in, full output out. Masks are all-ones (problem spec);
    bias is a scalar added to S pre-softmax, which cancels in both softmaxes."""
    global LAST_RESULT
    C = np.asarray(C, dtype=np.float32)
    in_maps = prep_in_maps(C, Q, w4C, w4Q, w4mlu)

    nc = get_nc()
    trace = bool(int(os.environ.get("BASS_KERNEL_TRACE", "0")))
    res = run_bass_kernel_spmd(nc, in_maps, list(range(NCORES)), trace=trace)
    LAST_RESULT = res

    # device OUT is [BPC, 128, NTC, 512] = [A|Bt|C*A|C*Bt]; row c = i*128 + p
    # lives at [p, i, :]; Bt is scratch (dropped here).
    acb = np.concatenate(
        [np.asarray(res.results[k]["OUT"]) for k in range(NCORES)],
        axis=0).astype(np.float32)                       # [B, 128, NTC, 512]
    acb = acb.transpose(0, 2, 1, 3).reshape(B, Lc, 4 * D)
    out = np.empty((B, Lc, 4 * D), dtype=np.float32)
    out[..., 0:D] = C
    out[..., D : 2 * D] = acb[..., 0:D]
    out[..., 2 * D :] = acb[..., 2 * D :]
    return out
